# revision 1
# baseline (speedup 1.0000x reference)
"""AdaptiveJacobianPrunedViT Trainium2 kernel: 8-core data-parallel, feature-major,
fp32r matmuls, masked token pruning with runtime scalars + per-layer AllReduce."""
import numpy as np
import concourse.bass as bass
import concourse.mybir as mybir
import concourse.tile as tile
from concourse import bacc
from concourse.bass_utils import run_bass_kernel_spmd

F32 = mybir.dt.float32
F32R = mybir.dt.float32r
I32 = mybir.dt.int32
AL = mybir.AluOpType
AF = mybir.ActivationFunctionType
AX = mybir.AxisListType

NCORES = 8
L = 12; D = 768; H = 12; HD = 64
NPATCH = 196; NT = 197; B = 32; BL = B // NCORES      # 4 imgs/core
TOK = BL * NT                                          # 788
CH = D // 128                                          # 6
SCALE = HD ** -0.5
GAMMA = 0.01; EPS = 1e-6
QPAD = 256                                             # padded query free dim
KC = [(0, 128), (128, 69)]                             # key-chunk (start, size)
HF = [(0, 394), (394, 394)]                            # token halves


def host_prep(inputs):
    """Rearrange weights/inputs into device layouts. Returns (shared, per_core)."""
    f = lambda a: np.ascontiguousarray(a, np.float32)
    qkv_w = f(inputs['qkv_w']); proj_w = f(inputs['proj_w'])
    fc1_w = f(inputs['fc1_w']); fc2_w = f(inputs['fc2_w'])

    def dout_slabs(w, dout):  # w [L, 768, dout] -> [L, dout//128, 128, 768]
        l, din, do = w.shape
        return np.ascontiguousarray(
            w.reshape(l, CH, 128, do // 128, 128).transpose(0, 3, 2, 1, 4).reshape(l, do // 128, 128, din))

    wq_h = dout_slabs(qkv_w[:, :, 0:768], 768)
    wk_h = dout_slabs(qkv_w[:, :, 768:1536], 768)
    wv_h = np.ascontiguousarray(qkv_w[:, :, 1536:2304].reshape(L, CH, 128, 768))  # din-slabs
    pj_h = dout_slabs(proj_w, 768)
    fc1_h = dout_slabs(fc1_w, 3072)                    # [L, 24, 128, 768]
    fc2_h = np.ascontiguousarray(                      # [L, 6, 128, 3072]
        fc2_w.reshape(L, 24, 128, CH, 128).transpose(0, 3, 2, 1, 4).reshape(L, CH, 128, 3072))
    wp = f(inputs['patch_w']).reshape(768, 768).T      # [in, out]
    wp_h = np.ascontiguousarray(
        wp.reshape(CH, 128, CH, 128).transpose(2, 1, 0, 3).reshape(CH, 128, 768))
    hw = np.zeros((768, 1024), np.float32); hw[:, :1000] = f(inputs['head_w'])
    hw_h = np.ascontiguousarray(hw.reshape(CH, 128, 8, 128).transpose(2, 1, 0, 3).reshape(8, 128, 768))

    def cols(v, n):  # [L, n*128] -> [L, 128, n]
        return np.ascontiguousarray(v.reshape(-1, n, 128).transpose(0, 2, 1))
    ln1w = cols(f(inputs['ln1_w']), CH); ln1b = cols(f(inputs['ln1_b']), CH)
    ln2w = cols(f(inputs['ln2_w']), CH); ln2b = cols(f(inputs['ln2_b']), CH)
    qkvb = f(inputs['qkv_b'])
    qb = cols(qkvb[:, 0:768], CH); kb = cols(qkvb[:, 768:1536], CH); vb_r = qkvb[:, 1536:2304]
    pjb = cols(f(inputs['proj_b']), CH); fc1b = cols(f(inputs['fc1_b']), 24)
    fc2b = cols(f(inputs['fc2_b']), CH)
    nw = cols(f(inputs['norm_w'])[None], CH)[0]; nb = cols(f(inputs['norm_b'])[None], CH)[0]
    hb = np.zeros((1024,), np.float32); hb[:1000] = f(inputs['head_b'])
    hb_h = cols(hb[None], 8)[0]

    LT = np.zeros((NT, NPATCH), np.float32)
    for t in range(1, NT):
        LT[t, :t - 1] = 1.0
    cones = np.zeros((128, 2), np.float32); cones[:, 0] = 1.0; cones[:, 1] = 1.0 / D

    pos = f(inputs['pos_embed'])[0]                    # [197, 768]
    cls = f(inputs['cls_token'])[0, 0]
    xaddT = np.zeros((768, NT), np.float32)
    xaddT[:, 0] = cls + pos[0]
    xaddT[:, 1:] = (pos[1:] + f(inputs['patch_b'])[None, :]).T
    xadd_core = np.tile(xaddT, (1, BL))                # [768, 788]
    arr = lambda A: np.ascontiguousarray(A.reshape(CH, 128, -1).transpose(1, 0, 2))

    x = f(inputs['x'])
    xcol_all = x.reshape(B, 3, 14, 16, 14, 16).transpose(0, 2, 4, 1, 3, 5).reshape(B, NPATCH, 768)
    per_core = []
    for c in range(NCORES):
        xcolT = np.zeros((768, TOK), np.float32)
        for i in range(BL):
            xcolT[:, NT * i + 1:NT * (i + 1)] = xcol_all[BL * c + i].T
        per_core.append({'xcol': arr(xcolT), 'xadd': arr(xadd_core)})
    shared = dict(wq=wq_h, wk=wk_h, wv=wv_h, pj=pj_h, fc1=fc1_h, fc2=fc2_h,
                  wp=wp_h, hw=hw_h, ln1w=ln1w, ln1b=ln1b, ln2w=ln2w, ln2b=ln2b,
                  qb=qb, kb=kb, vb=vb_r, pjb=pjb, fc1b=fc1b, fc2b=fc2b,
                  nw=nw[None], nb=nb[None], hb=hb_h[None], LT=LT, cones=cones)
    return shared, per_core


def build(n_layers=L):
    nc = bacc.Bacc("TRN2", target_bir_lowering=False, debug=False, num_devices=NCORES)
    P = {}
    def par(name, shape, dt):
        P[name] = nc.declare_dram_parameter(name, list(shape), dt, isOutput=False)
    par('wq', [L, CH, 128, 768], F32R); par('wk', [L, CH, 128, 768], F32R)
    par('wv', [L, CH, 128, 768], F32R); par('pj', [L, CH, 128, 768], F32R)
    par('fc1', [L, 24, 128, 768], F32R); par('fc2', [L, CH, 128, 3072], F32R)
    par('wp', [CH, 128, 768], F32R); par('hw', [8, 128, 768], F32)
    for n, w in [('ln1w', CH), ('ln1b', CH), ('ln2w', CH), ('ln2b', CH),
                 ('qb', CH), ('kb', CH), ('pjb', CH), ('fc1b', 24), ('fc2b', CH)]:
        par(n, [L, 128, w], F32)
    par('vb', [L, 768], F32)
    par('nw', [1, 128, CH], F32); par('nb', [1, 128, CH], F32); par('hb', [1, 128, 8], F32)
    par('LT', [NT, NPATCH], F32); par('cones', [128, 2], F32R)
    par('xcol', [128, CH, TOK], F32R); par('xadd', [128, CH, TOK], F32)
    out_ext = nc.declare_dram_parameter('out', [1024, BL], F32, isOutput=True)
    dbg_ext = nc.declare_dram_parameter('dbg', [1, 96], F32, isOutput=True)

    with tile.TileContext(nc) as tc:
        import contextlib
        ctx = contextlib.ExitStack()
        sb = ctx.enter_context(tc.tile_pool(name="sb", bufs=1))
        wpool = ctx.enter_context(tc.tile_pool(name="wp", bufs=3))
        w2pool = ctx.enter_context(tc.tile_pool(name="w2", bufs=3))
        xnp = ctx.enter_context(tc.tile_pool(name="xn", bufs=1))
        qim = ctx.enter_context(tc.tile_pool(name="qim", bufs=2))
        kim = ctx.enter_context(tc.tile_pool(name="kim", bufs=2))
        vim = ctx.enter_context(tc.tile_pool(name="vim", bufs=2))
        etp = ctx.enter_context(tc.tile_pool(name="etp", bufs=3))
        hhp = ctx.enter_context(tc.tile_pool(name="hh", bufs=1))
        scr = ctx.enter_context(tc.tile_pool(name="scr", bufs=1))
        wvp = ctx.enter_context(tc.tile_pool(name="wv", bufs=6))
        sqr = ctx.enter_context(tc.tile_pool(name="sqr", bufs=1))
        rkp = ctx.enter_context(tc.tile_pool(name="rk", bufs=1))
        bcp = ctx.enter_context(tc.tile_pool(name="bc", bufs=4))
        rowp = ctx.enter_context(tc.tile_pool(name="row", bufs=2))
        vbrp = ctx.enter_context(tc.tile_pool(name="vbr", bufs=1))
        rzp = ctx.enter_context(tc.tile_pool(name="rz", bufs=2))
        rzbp = ctx.enter_context(tc.tile_pool(name="rzb", bufs=2))
        vnp = ctx.enter_context(tc.tile_pool(name="vn", bufs=2))
        smp = ctx.enter_context(tc.tile_pool(name="sm", bufs=10))
        scp = ctx.enter_context(tc.tile_pool(name="sc", bufs=12))
        colp = ctx.enter_context(tc.tile_pool(name="col", bufs=12))
        pd = ctx.enter_context(tc.tile_pool(name="pd", bufs=5, space="PSUM"))
        ps2 = ctx.enter_context(tc.tile_pool(name="ps2", bufs=3, space="PSUM"))
        drp = ctx.enter_context(tc.tile_pool(name="dr", bufs=2, space="DRAM"))

        dma = nc.sync.dma_start
        V = nc.vector; S = nc.scalar; G = nc.gpsimd

        # ---- persistent tiles ----
        X = sb.tile([128, CH, TOK], F32, tag="X")
        cones_t = sb.tile([128, 2], F32R, tag="cones"); dma(cones_t[:], P['cones'][:])
        LT0 = sb.tile([128, NPATCH], F32, tag="LT0"); dma(LT0[:], P['LT'][0:128, :])
        LT1 = sb.tile([69, NPATCH], F32, tag="LT1"); dma(LT1[:], P['LT'][128:NT, :])
        m_t = [sb.tile([kn, 1], F32, tag=f"m{i}", name=f"m{i}") for i, (k0, kn) in enumerate(KC)]
        bias_t = [sb.tile([kn, 1], F32, tag=f"bias{i}", name=f"bias{i}") for i, (k0, kn) in enumerate(KC)]
        imp_t = [sb.tile([kn, 1], F32, tag=f"imp{i}", name=f"imp{i}") for i, (k0, kn) in enumerate(KC)]
        impcol = [sb.tile([kn, 1], F32, tag=f"impc{i}", name=f"impc{i}") for i, (k0, kn) in enumerate(KC)]
        imp_row = sb.tile([1, NT], F32, tag="improw")
        Np = sb.tile([1, 1], F32, tag="Np"); prevm = sb.tile([1, 1], F32, tag="prevm")
        hprev = sb.tile([1, 1], F32, tag="hprev")
        dbg = sb.tile([1, 96], F32, tag="dbg")
        for i in range(2):
            V.memset(m_t[i][:], 1.0); V.memset(bias_t[i][:], 0.0)
        V.memset(Np[:], float(NPATCH)); V.memset(prevm[:], 0.0); V.memset(hprev[:], 0.0)
        V.memset(dbg[:], 0.0)

        onesr = cones_t[:, 0:1]                    # f32r 1.0
        mean_l = cones_t[:, 1:2].bitcast(F32)      # f32 1/768

        def ln_into(xn_out, wcol, bcol):
            """LayerNorm of X -> xn_out (f32r), feature-axis, pipelined per token-half."""
            for hi, (h0, hn) in enumerate(HF):
                mu = rowp.tile([1, 394], F32, tag="mu", name=f"mu{hi}")
                sqm = rowp.tile([1, 394], F32, tag="sqm", name=f"sqm{hi}")
                pzx0 = ps2.tile([128, 2, QPAD], F32, tag="ps2")
                pzx = pzx0[0:1].rearrange("p a b -> p (a b)")[:, 0:394]
                for k in range(CH):
                    nc.tensor.matmul(pzx[0:1, 0:394], mean_l, X[:, k, h0:h0 + hn],
                                     start=(k == 0), stop=(k == CH - 1))
                V.tensor_copy(mu[0:1, :], pzx[0:1, 0:394])
                pzq0 = ps2.tile([128, 2, QPAD], F32, tag="ps2")
                pzq = pzq0[0:1].rearrange("p a b -> p (a b)")[:, 0:394]
                for k in range(CH):
                    sq = sqr.tile([128, 394], F32R, tag="sqr")
                    V.tensor_tensor(sq[:], X[:, k, h0:h0 + hn], X[:, k, h0:h0 + hn], AL.mult)
                    nc.tensor.matmul(pzq[0:1, 0:394], cones_t[:, 1:2], sq[:],
                                     start=(k == 0), stop=(k == CH - 1))
                V.tensor_copy(sqm[0:1, :], pzq[0:1, 0:394])
                tmp = rowp.tile([1, 394], F32, tag="tmp", name=f"tmp{hi}")
                V.tensor_tensor(tmp[:], mu[:], mu[:], AL.mult)
                V.tensor_tensor(sqm[:], sqm[:], tmp[:], AL.subtract)
                V.tensor_scalar_add(sqm[:], sqm[:], EPS)
                V.reciprocal(tmp[:], sqm[:])
                rstd = rowp.tile([1, 394], F32, tag="rstd", name=f"rstd{hi}")
                S.activation(rstd[:], tmp[:], AF.Sqrt)
                mu_b = bcp.tile([128, 394], F32, tag="bc", name=f"mub{hi}")
                rs_b = bcp.tile([128, 394], F32, tag="bc", name=f"rsb{hi}")
                G.partition_broadcast(mu_b[:], mu[:])
                G.partition_broadcast(rs_b[:], rstd[:])
                for k in range(CH):
                    t1 = scr.tile([128, TOK], F32, tag="scr")
                    V.tensor_tensor(t1[:, 0:394], X[:, k, h0:h0 + hn], mu_b[:], AL.subtract)
                    V.tensor_tensor(t1[:, 0:394], t1[:, 0:394], rs_b[:], AL.mult)
                    V.tensor_scalar(xn_out[:, k, h0:h0 + hn], t1[:, 0:394], wcol[:, k:k + 1], bcol[:, k:k + 1],
                                    AL.mult, AL.add)

        def dense_from(xn_t, wslab_param, li, nchunk_out, bias_col, dst_write, halves=(0, 1)):
            """out chunks j: psum = sum_k wslab[j][:,k*128:...] ^T @ xn[:,k,half]; dst_write(j, hi, psum)."""
            for j in range(nchunk_out):
                wt = wpool.tile([128, 768], F32R, tag="w768")
                dma(wt[:], wslab_param[li, j] if li is not None else wslab_param[j])
                for hi in halves:
                    h0, hn = HF[hi]
                    pt = pd.tile([128, 394], F32, tag="pd")
                    for k in range(CH):
                        nc.tensor.matmul(pt[:], wt[:, k * 128:(k + 1) * 128], xn_t[:, k, h0:h0 + hn],
                                         start=(k == 0), stop=(k == CH - 1))
                    dst_write(j, hi, pt)

        # ---- patch embed ----
        dma(X[:], P['xadd'][:])
        xcol_t = xnp.tile([128, CH, TOK], F32R, tag="xn")
        dma(xcol_t[:], P['xcol'][:])

        def patch_write(j, hi, pt):
            h0, hn = HF[hi]
            V.scalar_tensor_tensor(X[:, j, h0:h0 + hn], pt[:], 0.0, X[:, j, h0:h0 + hn], AL.add, AL.add)
        dense_from(xcol_t, P['wp'], None, CH, None, patch_write)

        # ---- layers ----
        for li in range(n_layers):
            # LN1
            lw = colp.tile([128, CH], F32, tag="colc"); dma(lw[:], P['ln1w'][li])
            lb = colp.tile([128, CH], F32, tag="colc"); dma(lb[:], P['ln1b'][li])
            xn_t = xnp.tile([128, CH, TOK], F32R, tag="xn")
            ln_into(xn_t, lw, lb)

            # Q, K, V + attention, processed per token-half (2 images each)
            qbc = colp.tile([128, CH], F32, tag="colc"); dma(qbc[:], P['qb'][li])
            kbc = colp.tile([128, CH], F32, tag="colc"); dma(kbc[:], P['kb'][li])
            vbr = vbrp.tile([1, 768], F32, tag="vbr"); dma(vbr[:], P['vb'][li].unsqueeze(0))
            VB = bcp.tile([128, 768], F32, tag="vbt")
            G.partition_broadcast(VB[:], vbr[:])
            wvts = []
            for k in range(CH):
                wvt = wvp.tile([128, 768], F32R, tag="wv6", name=f"wv{li}_{k}")
                dma(wvt[:], P['wv'][li, k])
                wvts.append(wvt)
            OT = hhp.tile([128, CH, TOK], F32R, tag="hh", name=f"OT{li}")
            for i in range(2):
                V.memset(imp_t[i][:], 0.0)
            for hi2 in range(2):
                Qi = [qim.tile([128, CH, QPAD], F32R, tag="qim", name=f"Qi{li}_{hi2}_{_}") for _ in range(2)]
                Ki = [kim.tile([128, CH, NT], F32R, tag="kim", name=f"Ki{li}_{hi2}_{_}") for _ in range(2)]
                Vi = [vim.tile([128, 2, 864], F32R, tag="vim", name=f"Vi{li}_{hi2}_{_}") for _ in range(2)]
                for ii in range(2):
                    S.activation(Qi[ii][:, :, NT:QPAD], X[:, :, 0:QPAD - NT], AF.Copy, scale=0.0)

                def q_write(j, hi, pt):
                    for ii in range(2):
                        S.activation(Qi[ii][:, j, 0:NT], pt[:, NT * ii:NT * ii + NT],
                                     AF.Identity, bias=qbc[:, j:j + 1], scale=1.0)
                dense_from(xn_t, P['wq'], li, CH, None, q_write, halves=(hi2,))

                def k_write(j, hi, pt):
                    for ii in range(2):
                        S.activation(Ki[ii][:, j, 0:NT], pt[:, NT * ii:NT * ii + NT],
                                     AF.Identity, bias=kbc[:, j:j + 1], scale=1.0)
                dense_from(xn_t, P['wk'], li, CH, None, k_write, halves=(hi2,))

                for ii in range(2):
                    img = 2 * hi2 + ii
                    for kc, (k0, kn) in enumerate(KC):
                        c0 = NT * img + k0
                        for dh in range(2):
                            pv = pd.tile([128, 394], F32, tag="pd")
                            for k in range(CH):
                                nc.tensor.matmul(pv[0:kn, 0:384], xn_t[:, k, c0:c0 + kn],
                                                 wvts[k][:, dh * 384:(dh + 1) * 384],
                                                 start=(k == 0), stop=(k == CH - 1))
                            vdst = Vi[ii][0:kn, kc, dh * 432:dh * 432 + 432].rearrange(
                                "p (h c) -> p h c", c=72)[:, :, 0:64]
                            V.tensor_tensor(vdst,
                                            pv[0:kn, 0:384].rearrange("p (h c) -> p h c", c=64),
                                            VB[0:kn, dh * 384:(dh + 1) * 384].rearrange("p (h c) -> p h c", c=64),
                                            AL.add)
                        for kc, (k0, kn) in enumerate(KC):
                            vone = Vi[ii][0:kn, kc, :].rearrange("p (h c) -> p h c", c=72)[:, :, 64:65]
                            S.activation(vone, X[0:kn, 0, 0:H].unsqueeze(2), AF.Identity,
                                         bias=cones_t[0:kn, 0:1].bitcast(F32), scale=0.0)

                for ii in range(2):
                    img = 2 * hi2 + ii
                    vn = vnp.tile([128, 2, H], F32, tag="vn", name=f"vn{li}_{img}")
                    vns = vnp.tile([128, 2, H], F32, tag="vns", name=f"vns{li}_{img}")
                    for kc, (k0, kn) in enumerate(KC):
                        sqv = scr.tile([128, TOK], F32, tag="scr", name=f"sqv{li}_{img}_{kc}")
                        vsl = Vi[ii][0:kn, kc, :].rearrange("p (h c) -> p h c", c=72)[:, :, 0:64].bitcast(F32)
                        V.tensor_tensor(sqv[0:kn, 0:768].rearrange("p (h c) -> p h c", c=64),
                                        vsl, vsl, AL.mult)
                        for h in range(H):
                            V.tensor_reduce(vn[0:kn, kc, h:h + 1], sqv[0:kn, h * HD:(h + 1) * HD],
                                            AX.XYZW, AL.add)
                        S.activation(vns[0:kn, kc, :], vn[0:kn, kc, :], AF.Sqrt)
                    for h in range(H):
                        hb_, hc = (h % 2) * 64, h // 2
                        ps = ps2.tile([128, 2, QPAD], F32, tag="ps2")
                        for kc, (k0, kn) in enumerate(KC):
                            nc.tensor.matmul(ps[0:kn, kc, :], Ki[ii][hb_:hb_ + 64, hc, k0:k0 + kn],
                                             Qi[ii][hb_:hb_ + 64, hc, :], start=True, stop=True)
                        et = etp.tile([128, 2, QPAD], F32R, tag="etp")
                        for kc, (k0, kn) in enumerate(KC):
                            S.activation(et[0:kn, kc, :], ps[0:kn, kc, :], AF.Exp,
                                         bias=bias_t[kc][0:kn], scale=SCALE)
                        pav2 = pd.tile([128, 394], F32, tag="pd")
                        pav = pav2[:, 0:QPAD]
                        for kc, (k0, kn) in enumerate(KC):
                            nc.tensor.matmul(pav[0:65, 0:QPAD], Vi[ii][0:kn, kc, 72 * h:72 * h + 65],
                                             et[0:kn, kc, :], start=(kc == 0), stop=(kc == 1))
                        rz = rzp.tile([1, QPAD], F32, tag="rz")
                        V.reciprocal(rz[:], pav[64:65, 0:QPAD])
                        rzb = rzbp.tile([64, QPAD], F32, tag="rzb")
                        G.partition_broadcast(rzb[:], rz[:])
                        V.tensor_tensor(OT[hb_:hb_ + 64, hc, NT * img:NT * img + NT],
                                        pav[0:64, 0:NT], rzb[0:64, 0:NT], AL.mult)
                        if li < n_layers - 1:
                            r0b = smp.tile([128, 1], F32, tag="sm")
                            G.partition_broadcast(r0b[:], rz[0:1, 0:1])
                            for kc, (k0, kn) in enumerate(KC):
                                tv = smp.tile([128, 1], F32, tag="sm")
                                V.tensor_tensor(tv[0:kn, :], et[0:kn, kc, 0:1].bitcast(F32),
                                                vns[0:kn, kc, h:h + 1], AL.mult)
                                V.scalar_tensor_tensor(imp_t[kc][0:kn, :], tv[0:kn, :], r0b[0:kn, :],
                                                       imp_t[kc][0:kn, :], AL.mult, AL.add)

            # importance AllReduce + stats + mask update (skip after last layer)
            if li < n_layers - 1:
                arin = drp.tile([NT, 1], F32, tag="arin")
                arout = drp.tile([NT, 1], F32, tag="arout")
                for kc, (k0, kn) in enumerate(KC):
                    dma(arin[k0:k0 + kn, :], imp_t[kc][:])
                G.collective_compute("AllReduce", AL.add,
                                     replica_groups=[list(range(NCORES))],
                                     ins=[arin[:].opt()], outs=[arout[:].opt()])
                dma(imp_row[0:1, :], arout[:].transpose([1, 0]))
                for kc, (k0, kn) in enumerate(KC):
                    dma(impcol[kc][:], arout[k0:k0 + kn, :])
                V.tensor_scalar_mul(imp_row[:], imp_row[:], 1.0 / (B * H))
                for kc in range(2):
                    V.tensor_scalar_mul(impcol[kc][:], impcol[kc][:], 1.0 / (B * H))
                V.memset(imp_row[0:1, 0:1], 1e30)
                V.memset(impcol[0][0:1, :], 1e30)
                impp = imp_row[0:1, 1:NT]
                # scalars
                mass = scp.tile([1, 1], F32, tag="sc")
                V.tensor_reduce(mass[:], impp, AX.XYZW, AL.add)
                sqrow = rowp.tile([1, NPATCH], F32, tag="sqrow")
                V.tensor_tensor(sqrow[:], impp, impp, AL.mult)
                ssq = scp.tile([1, 1], F32, tag="sc")
                V.tensor_reduce(ssq[:], sqrow[:], AX.XYZW, AL.add)
                rNp = scp.tile([1, 1], F32, tag="sc"); V.reciprocal(rNp[:], Np[:])
                mean = scp.tile([1, 1], F32, tag="sc"); V.tensor_tensor(mean[:], mass[:], rNp[:], AL.mult)
                var = scp.tile([1, 1], F32, tag="sc")
                V.tensor_tensor(var[:], ssq[:], rNp[:], AL.mult)
                t0 = scp.tile([1, 1], F32, tag="sc"); V.tensor_tensor(t0[:], mean[:], mean[:], AL.mult)
                V.tensor_tensor(var[:], var[:], t0[:], AL.subtract)
                sdev = scp.tile([1, 1], F32, tag="sc"); S.activation(sdev[:], var[:], AF.Sqrt)
                me = scp.tile([1, 1], F32, tag="sc"); V.tensor_scalar_add(me[:], mean[:], EPS)
                rme = scp.tile([1, 1], F32, tag="sc"); V.reciprocal(rme[:], me[:])
                rho = scp.tile([1, 1], F32, tag="sc"); V.tensor_tensor(rho[:], sdev[:], rme[:], AL.mult)
                dr = scp.tile([1, 1], F32, tag="sc"); V.tensor_tensor(dr[:], mass[:], prevm[:], AL.subtract)
                drn = scp.tile([1, 1], F32, tag="sc"); V.tensor_scalar_mul(drn[:], dr[:], -1.0)
                V.tensor_tensor(dr[:], dr[:], drn[:], AL.max)
                pe = scp.tile([1, 1], F32, tag="sc"); V.tensor_scalar_add(pe[:], prevm[:], EPS)
                rpe = scp.tile([1, 1], F32, tag="sc"); V.reciprocal(rpe[:], pe[:])
                V.tensor_tensor(dr[:], dr[:], rpe[:], AL.mult)
                V.tensor_scalar_add(dr[:], dr[:], 1.0)
                kr = scp.tile([1, 1], F32, tag="sc"); V.tensor_tensor(kr[:], rho[:], dr[:], AL.mult)
                V.tensor_scalar(kr[:], kr[:], -GAMMA, 1.0, AL.mult, AL.add)
                V.tensor_scalar_max(kr[:], kr[:], 0.0)
                nk = scp.tile([1, 1], F32, tag="sc"); V.tensor_tensor(nk[:], Np[:], kr[:], AL.mult)
                V.tensor_scalar_add(nk[:], nk[:], -0.5)
                nki = scp.tile([1, 1], I32, tag="sci"); V.tensor_copy(nki[:], nk[:])
                nnext = scp.tile([1, 1], F32, tag="sc"); V.tensor_copy(nnext[:], nki[:])
                V.tensor_scalar_max(nnext[:], nnext[:], 16.0)
                skip = scp.tile([1, 1], F32, tag="sc")
                V.tensor_scalar(skip[:], Np[:], 16.5, None, AL.is_lt)
                nskip = scp.tile([1, 1], F32, tag="sc")
                V.tensor_scalar(nskip[:], skip[:], -1.0, 1.0, AL.mult, AL.add)
                dp = scp.tile([1, 1], F32, tag="sc"); V.tensor_tensor(dp[:], nskip[:], hprev[:], AL.mult)
                lt2 = scp.tile([1, 1], F32, tag="sc"); V.tensor_tensor(lt2[:], nnext[:], Np[:], AL.is_lt)
                V.tensor_tensor(dp[:], dp[:], lt2[:], AL.mult)
                V.tensor_copy(hprev[:], nskip[:])
                V.tensor_copy(prevm[:], mass[:])
                tn = scp.tile([1, 1], F32, tag="sc"); V.tensor_tensor(tn[:], nnext[:], Np[:], AL.subtract)
                V.scalar_tensor_tensor(Np[:], tn[:], dp[:], Np[:], AL.mult, AL.add)
                # debug
                if li < 12:
                    for slot, src in enumerate([mass, rho, nnext, Np, dp, kr]):
                        V.tensor_copy(dbg[0:1, 8 * li + slot:8 * li + slot + 1], src[:])
                # mask update
                dpb = smp.tile([128, 1], F32, tag="sm"); G.partition_broadcast(dpb[:], dp[:])
                kb_ = smp.tile([128, 1], F32, tag="sm"); G.partition_broadcast(kb_[:], nnext[:])
                impb = rkp.tile([128, NPATCH], F32, tag="impb")
                G.partition_broadcast(impb[:], impp)
                LTt = [LT0, LT1]
                for kc, (k0, kn) in enumerate(KC):
                    cg = rkp.tile([128, NPATCH], F32, tag="cg")
                    V.tensor_scalar(cg[0:kn, :], impb[0:kn, :], impcol[kc][:], None, AL.is_gt)
                    rank = smp.tile([128, 1], F32, tag="sm")
                    V.tensor_reduce(rank[0:kn, :], cg[0:kn, :], AX.XYZW, AL.add)
                    ce = rkp.tile([128, NPATCH], F32, tag="cg")
                    V.tensor_scalar(ce[0:kn, :], impb[0:kn, :], impcol[kc][:], None, AL.is_equal)
                    V.tensor_tensor(ce[0:kn, :], ce[0:kn, :], LTt[kc][0:kn, :], AL.mult)
                    req = smp.tile([128, 1], F32, tag="sm")
                    V.tensor_reduce(req[0:kn, :], ce[0:kn, :], AX.XYZW, AL.add)
                    V.tensor_tensor(rank[0:kn, :], rank[0:kn, :], req[0:kn, :], AL.add)
                    mc = smp.tile([128, 1], F32, tag="sm")
                    V.tensor_scalar(mc[0:kn, :], rank[0:kn, :], kb_[0:kn, :], None, AL.is_lt)
                    V.tensor_tensor(mc[0:kn, :], mc[0:kn, :], m_t[kc][0:kn, :], AL.subtract)
                    V.scalar_tensor_tensor(m_t[kc][0:kn, :], mc[0:kn, :], dpb[0:kn, :],
                                           m_t[kc][0:kn, :], AL.mult, AL.add)
                    V.tensor_scalar(bias_t[kc][0:kn, :], m_t[kc][0:kn, :], 1.0, 1e5,
                                    AL.subtract, AL.mult)

            # proj + residual
            pjbc = colp.tile([128, CH], F32, tag="colc"); dma(pjbc[:], P['pjb'][li])

            def pj_write(j, hi, pt):
                h0, hn = HF[hi]
                V.scalar_tensor_tensor(X[:, j, h0:h0 + hn], pt[:], pjbc[:, j:j + 1],
                                       X[:, j, h0:h0 + hn], AL.add, AL.add)
            dense_from(OT, P['pj'], li, CH, None, pj_write)

            # LN2 + MLP
            l2w = colp.tile([128, CH], F32, tag="colc"); dma(l2w[:], P['ln2w'][li])
            l2b = colp.tile([128, CH], F32, tag="colc"); dma(l2b[:], P['ln2b'][li])
            xn2 = xnp.tile([128, CH, TOK], F32R, tag="xn")
            ln_into(xn2, l2w, l2b)
            f1b = colp.tile([128, 24], F32, tag="colc"); dma(f1b[:], P['fc1b'][li])
            f2b = colp.tile([128, CH], F32, tag="colc"); dma(f2b[:], P['fc2b'][li])
            for hi, (h0, hn) in enumerate(HF):
                Ht = hhp.tile([128, 24, 394], F32R, tag="hh")
                for jj in range(24):
                    w1 = wpool.tile([128, 768], F32R, tag="w768")
                    dma(w1[:], P['fc1'][li, jj])
                    ph = pd.tile([128, 394], F32, tag="pd")
                    for k in range(CH):
                        nc.tensor.matmul(ph[:], w1[:, k * 128:(k + 1) * 128], xn2[:, k, h0:h0 + hn],
                                         start=(k == 0), stop=(k == CH - 1))
                    S.activation(Ht[:, jj, :], ph[:], AF.Gelu, bias=f1b[:, jj:jj + 1], scale=1.0)
                for j in range(CH):
                    w2s = []
                    for t3 in range(3):
                        w2t = w2pool.tile([128, 1024], F32R, tag="wfc2", name=f"w2{li}_{hi}_{j}_{t3}")
                        dma(w2t[:], P['fc2'][li, j][:, 1024 * t3:1024 * (t3 + 1)])
                        w2s.append(w2t)
                    pf = pd.tile([128, 394], F32, tag="pd")
                    for k in range(24):
                        wsl = w2s[k // 8]
                        nc.tensor.matmul(pf[:], wsl[:, (k % 8) * 128:(k % 8 + 1) * 128], Ht[:, k, :],
                                         start=(k == 0), stop=(k == 23))
                    V.scalar_tensor_tensor(X[:, j, h0:h0 + hn], pf[:], f2b[:, j:j + 1],
                                           X[:, j, h0:h0 + hn], AL.add, AL.add)

        # ---- final LN (CLS columns only) + head ----
        Xc = sb.tile([128, CH, BL], F32, tag="Xc")
        for k in range(CH):
            for i in range(BL):
                V.tensor_copy(Xc[:, k, i:i + 1], X[:, k, NT * i:NT * i + 1])
        pzc0 = ps2.tile([128, 2, QPAD], F32, tag="ps2")
        pzc = pzc0[0:1].rearrange("p a b -> p (a b)")[:, 0:394]
        for k in range(CH):
            nc.tensor.matmul(pzc[0:1, 0:BL], mean_l, Xc[:, k, :],
                             start=(k == 0), stop=(k == CH - 1))
        muc = rowp.tile([1, BL], F32, tag="mu"); V.tensor_copy(muc[:], pzc[0:1, 0:BL])
        sqc = sb.tile([128, CH, BL], F32, tag="sqc")
        V.tensor_tensor(sqc[:], Xc[:], Xc[:], AL.mult)
        pzq20 = ps2.tile([128, 2, QPAD], F32, tag="ps2")
        pzq2 = pzq20[0:1].rearrange("p a b -> p (a b)")[:, 0:394]
        for k in range(CH):
            nc.tensor.matmul(pzq2[0:1, 0:BL], mean_l, sqc[:, k, :],
                             start=(k == 0), stop=(k == CH - 1))
        varc = rowp.tile([1, BL], F32, tag="sqm")
        t2 = rowp.tile([1, BL], F32, tag="tmp")
        V.tensor_tensor(t2[:], muc[:], muc[:], AL.mult)
        V.tensor_copy(varc[:], pzq2[0:1, 0:BL])
        V.tensor_tensor(varc[:], varc[:], t2[:], AL.subtract)
        V.tensor_scalar_add(varc[:], varc[:], EPS)
        recc = vbrp.tile([1, BL], F32, tag="vbr"); V.reciprocal(recc[:], varc[:])
        rstc = rowp.tile([1, BL], F32, tag="rstd"); S.activation(rstc[:], recc[:], AF.Sqrt)
        mucb = smp.tile([128, BL], F32, tag="smb"); G.partition_broadcast(mucb[:], muc[:])
        rscb = smp.tile([128, BL], F32, tag="smb"); G.partition_broadcast(rscb[:], rstc[:])
        nwc = sb.tile([128, CH], F32, tag="nwc"); dma(nwc[:], P['nw'][0])
        nbc = sb.tile([128, CH], F32, tag="nbc"); dma(nbc[:], P['nb'][0])
        Xnc = sb.tile([128, CH, BL], F32, tag="Xnc")
        for k in range(CH):
            tt_ = smp.tile([128, BL], F32, tag="smb")
            V.tensor_tensor(tt_[:], Xc[:, k, :], mucb[:], AL.subtract)
            V.tensor_tensor(tt_[:], tt_[:], rscb[:], AL.mult)
            V.tensor_scalar(Xnc[:, k, :], tt_[:], nwc[:, k:k + 1], nbc[:, k:k + 1], AL.mult, AL.add)
        hbc = sb.tile([128, 8], F32, tag="hbc"); dma(hbc[:], P['hb'][0])
        outsb = sb.tile([128, 8, BL], F32, tag="outsb")
        for j in range(8):
            wh = wpool.tile([128, 768], F32, tag="w768")
            dma(wh[:], P['hw'][j])
            po_ = pd.tile([128, 394], F32, tag="pd")
            for k in range(CH):
                nc.tensor.matmul(po_[0:128, 0:BL], wh[:, k * 128:(k + 1) * 128], Xnc[:, k, :],
                                 start=(k == 0), stop=(k == CH - 1))
            V.tensor_scalar(outsb[:, j, :], po_[0:128, 0:BL], hbc[:, j:j + 1], None, AL.add)
            dma(out_ext[128 * j:128 * (j + 1), :], outsb[:, j, :])
        dma(dbg_ext[:], dbg[:])
        ctx.close()
    nc.compile()
    return nc


def run(inputs, n_layers=L, trace=False):
    shared, per_core = host_prep(inputs)
    nc = build(n_layers)
    in_maps = []
    for c in range(NCORES):
        m = dict(shared); m.update(per_core[c])
        in_maps.append(m)
    res = run_bass_kernel_spmd(nc, in_maps, list(range(NCORES)), trace=trace)
    outs = []
    for c in range(NCORES):
        outs.append(res.results[c]['out'][:1000, :].T)   # [4, 1000]
    full = np.concatenate(outs, axis=0)
    return full, res


def kernel(**inputs):
    """Harness entry: full inputs -> full [32, 1000] output, computed on 8 NeuronCores."""
    full, _res = run(inputs, n_layers=L, trace=False)
    return np.ascontiguousarray(full, dtype=np.float32)

